# revision 1
# baseline (speedup 1.0000x reference)
"""BiMPM MatchingLayer kernel for Trainium2, 8 NeuronCores, batch-data-parallel.

Full inputs: p (32,64,600), q (32,64,600), W (8,20,300).
Output: tuple (mv_p, mv_q), each (32,64,160).

Per core: 4 batches x 2 directions (fw: cols 0:300 of p/q, bw: cols 300:600).
All cosine matchings are computed from transposed (h-on-partitions) layouts so
per-row normalizations are per-partition scalars.
"""

import numpy as np

S, H, L, NB, NCORES = 64, 300, 20, 4, 8
CH = [(0, 128), (128, 256), (256, 300)]
WL = 8 * L

# tunables
F32R_REP = False    # use float32r for ones-replication matmuls (validated exact-or-not by smoke)
F32R_MM = False     # use float32r for the big maxpool matmuls (N=512 chunks)
REP_BLK = 1024     # free-size of each C-replication block (s-or-t block of REP_BLK//64)

_CACHE = {}


def _bc_mid(bassmod, ap2, n, pos):
    """Insert a stride-0 broadcast dim of count n into a 2D AP's free dims.
    pos=0: (p, f) -> (p, n, f); pos=1: (p, f) -> (p, f, n)."""
    a = list(ap2.ap)
    assert len(a) == 2, a
    if pos == 0:
        new = [a[0], [0, n], a[1]]
    else:
        new = [a[0], a[1], [0, n]]
    return bassmod.AP(tensor=ap2.tensor, offset=ap2.offset, ap=new)


def _build(nb=NB, en=("fu", "mp", "am", "ax")):
    import concourse.bass as bass
    import concourse.tile as tile
    from concourse import bacc, mybir
    from concourse.masks import make_identity
    from contextlib import ExitStack

    f32 = mybir.dt.float32
    f32r = mybir.dt.float32r
    AX = mybir.AxisListType
    OPT = mybir.AluOpType

    def r32(ap):
        return ap.bitcast(f32r)

    nc = bacc.Bacc("TRN2", target_bir_lowering=False, debug=False,
                   enable_asserts=False, num_devices=NCORES)
    p_d = nc.dram_tensor("p", [nb, S, 2 * H], f32, kind="ExternalInput").ap()
    q_d = nc.dram_tensor("q", [nb, S, 2 * H], f32, kind="ExternalInput").ap()
    w_d = nc.dram_tensor("W", [8, L, H], f32, kind="ExternalInput").ap()
    op_d = nc.dram_tensor("op", [nb, S, WL], f32, kind="ExternalOutput").ap()
    oq_d = nc.dram_tensor("oq", [nb, S, WL], f32, kind="ExternalOutput").ap()

    with tile.TileContext(nc) as tc, ExitStack() as ctx:
        const = ctx.enter_context(tc.tile_pool(name="const", bufs=1))
        sb = ctx.enter_context(tc.tile_pool(name="sb", bufs=2))
        sbx = ctx.enter_context(tc.tile_pool(name="sbx", bufs=2))
        sbX = ctx.enter_context(tc.tile_pool(name="sbX", bufs=4))
        sb3 = ctx.enter_context(tc.tile_pool(name="sb3", bufs=3))
        ps = ctx.enter_context(tc.tile_pool(name="ps", bufs=3, space="PSUM"))
        psdl = ctx.enter_context(tc.tile_pool(name="psdl", bufs=1, space="PSUM"))
        psrep = ctx.enter_context(tc.tile_pool(name="psrep", bufs=1, space="PSUM"))
        dram = ctx.enter_context(tc.tile_pool(name="dram", bufs=2, space="DRAM"))

        ident = const.tile([128, 128], f32, tag="ident")
        make_identity(nc, ident)
        ones = const.tile([1, 128], f32, tag="ones")
        nc.vector.memset(ones[:], 1.0)

        # ---- W precompute: VTall[hp, ci, w*L + l] = W[w, l, h0+hp]^2 ----
        vtall = const.tile([128, 3, WL], f32, tag="vtall")
        for w in range(8):
            wt = sb.tile([L, H], f32, tag="wt")
            nc.sync.dma_start(wt[:], w_d[w])
            v2 = sb.tile([L, H], f32, tag="v2")
            nc.vector.tensor_mul(v2[:], wt[:], wt[:])
            for ci, (h0, h1) in enumerate(CH):
                hc = h1 - h0
                pt = ps.tile([128, 192], f32, tag="t")
                nc.tensor.transpose(pt[:hc, 0:L], v2[:, h0:h1], ident[0:L, 0:L])
                nc.scalar.copy(vtall[:hc, ci, w * L:(w + 1) * L], pt[:hc, 0:L])

        def vts(ci, w):
            return vtall[:CH[ci][1] - CH[ci][0], ci, w * L:(w + 1) * L]

        def transpose_to(dst3, src2d, rows):
            """src2d (rows, 300) sbuf -> dst3 (128,3,rows) chunked transpose."""
            for ci, (h0, h1) in enumerate(CH):
                hc = h1 - h0
                pt = ps.tile([128, 192], f32, tag="t")
                nc.tensor.transpose(pt[:hc, 0:rows], src2d[:, h0:h1], ident[0:rows, 0:rows])
                nc.scalar.copy(dst3[:hc, ci, :], pt[:hc, 0:rows])

        def flat3(t3, hc, ci):
            """(128,3,A,B) tile -> (hc, A*B) 2D AP for chunk ci."""
            ap = t3[:hc, ci]
            a = list(ap.ap)
            # merge trailing dims (contiguous)
            n = 1
            for st, ct in a[1:]:
                n *= ct
            return bass.AP(tensor=ap.tensor, offset=ap.offset, ap=[a[0], [1, n]])

        def mp_tail(numps, y_t, w, invnAll, side_out, sgn=None, clamp=True):
            """Common mp_cos tail: given num (64,L) psum and transposed y (128,3,S),
            compute den from y^2 @ V_w, combine, write to side_out slice."""
            y2 = sb.tile([128, 3, S], f32, tag="y2")
            nc.vector.tensor_mul(y2[:], y_t[:], y_t[:])
            denps = ps.tile([128, 192], f32, tag="t")
            for ci, (h0, h1) in enumerate(CH):
                hc = h1 - h0
                nc.tensor.matmul(denps[0:S, 0:L], y2[:hc, ci, :], vts(ci, w),
                                 start=(ci == 0), stop=(ci == 2))
            ny = sb.tile([S, L], f32, tag="ny")
            nc.scalar.sqrt(ny[:], denps[0:S, 0:L])
            invy = sb.tile([S, L], f32, tag="invy")
            scr = sb.tile([S, L], f32, tag="scrL")
            nc.vector.reciprocal_approx_accurate(invy[:], ny[:], scr[:])
            c1 = sb.tile([S, L], f32, tag="c1")
            nc.vector.tensor_mul(c1[:], invnAll[:, w * L:(w + 1) * L], invy[:])
            if clamp:
                c2 = sb.tile([S, L], f32, tag="c2")
                nc.vector.tensor_scalar_min(c2[:], c1[:], 1e8)
                c1 = c2
            c3 = sb.tile([S, L], f32, tag="c3")
            nc.vector.tensor_mul(c3[:], numps[0:S, 0:L], c1[:])
            if sgn is not None:
                nc.vector.tensor_scalar_mul(side_out[:, w * L:(w + 1) * L], c3[:], sgn[:])
            else:
                nc.vector.tensor_copy(side_out[:, w * L:(w + 1) * L], c3[:])

        for b in range(nb):
            opt = sb.tile([S, WL], f32, tag="OP")
            oqt = sb.tile([S, WL], f32, tag="OQ")
            if len(en) < 4:
                nc.gpsimd.memset(opt[:], 0.0)
                nc.gpsimd.memset(oqt[:], 0.0)
            for d in range(2):
                c0 = d * H
                P = sb.tile([S, H], f32, tag="P")
                nc.sync.dma_start(P[:], p_d[b, :, c0:c0 + H])
                Q = sb.tile([S, H], f32, tag="Q")
                nc.sync.dma_start(Q[:], q_d[b, :, c0:c0 + H])

                PT = sb3.tile([128, 3, S], f32, tag="PT")
                transpose_to(PT, P, S)
                QT = sb3.tile([128, 3, S], f32, tag="QT")
                transpose_to(QT, Q, S)
                PT2 = sb3.tile([128, 3, S], f32, tag="PT2")
                nc.vector.tensor_mul(PT2[:], PT[:], PT[:])
                QT2 = sb3.tile([128, 3, S], f32, tag="QT2")
                nc.vector.tensor_mul(QT2[:], QT[:], QT[:])

                # ---- plain row norms + normalized cosine matrix ----
                scr300 = sb.tile([S, H], f32, tag="scr300")
                nsq = sb.tile([S, 1], f32, tag="nsqP")
                nc.vector.tensor_mul(scr300[:], P[:], P[:])
                nc.vector.reduce_sum(out=nsq[:], in_=scr300[:], axis=AX.X)
                invnP = sb.tile([S, 1], f32, tag="invnP")
                nP = sb.tile([S, 1], f32, tag="nP")
                nc.scalar.sqrt(nP[:], nsq[:])
                nc.vector.reciprocal(invnP[:], nP[:])
                nsqQ = sb.tile([S, 1], f32, tag="nsqQ")
                scr300b = sb.tile([S, H], f32, tag="scr300b")
                nc.vector.tensor_mul(scr300b[:], Q[:], Q[:])
                nc.vector.reduce_sum(out=nsqQ[:], in_=scr300b[:], axis=AX.X)
                invnQ = sb.tile([S, 1], f32, tag="invnQ")
                nQ = sb.tile([S, 1], f32, tag="nQ")
                nc.scalar.sqrt(nQ[:], nsqQ[:])
                nc.vector.reciprocal(invnQ[:], nQ[:])

                Qn = sb.tile([S, H], f32, tag="Qn")
                nc.vector.tensor_scalar_mul(Qn[:], Q[:], invnQ[:])
                QnT = sb3.tile([128, 3, S], f32, tag="QnT")
                transpose_to(QnT, Qn, S)

                cut = ps.tile([128, 192], f32, tag="t")
                for ci, (h0, h1) in enumerate(CH):
                    hc = h1 - h0
                    nc.tensor.matmul(cut[0:S, 0:S], QnT[:hc, ci, :], PT[:hc, ci, :],
                                     start=(ci == 0), stop=(ci == 2))
                cut_sb = sb.tile([S, S], f32, tag="cut_sb")
                nc.vector.tensor_copy(cut_sb[:], cut[0:S, 0:S])
                cu = ps.tile([128, 192], f32, tag="t")
                nc.tensor.transpose(cu[0:S, 0:S], cut_sb[:], ident[0:S, 0:S])
                Cs = sb3.tile([S, S], f32, tag="Cs")
                nc.vector.tensor_scalar_mul(Cs[:], cu[0:S, 0:S], invnP[:])
                ctp = ps.tile([128, 192], f32, tag="t")
                nc.tensor.transpose(ctp[0:S, 0:S], Cs[:], ident[0:S, 0:S])
                Ct = sb3.tile([S, S], f32, tag="Ct")
                nc.scalar.copy(Ct[:], ctp[0:S, 0:S])

                # ---- weighted norms, all 8 perspectives: (64, 160) ----
                p2v = ps.tile([128, 192], f32, tag="t")
                for ci, (h0, h1) in enumerate(CH):
                    hc = h1 - h0
                    nc.tensor.matmul(p2v[0:S, 0:WL], PT2[:hc, ci, :], vtall[:hc, ci, :],
                                     start=(ci == 0), stop=(ci == 2))
                invnpAll = sb.tile([S, WL], f32, tag="invnpAll")
                npw = sb.tile([S, WL], f32, tag="npw")
                nc.scalar.sqrt(npw[:], p2v[0:S, 0:WL])
                scrW = sb.tile([S, WL], f32, tag="scrW")
                nc.vector.reciprocal_approx_accurate(invnpAll[:], npw[:], scrW[:])
                q2v = ps.tile([128, 192], f32, tag="t")
                for ci, (h0, h1) in enumerate(CH):
                    hc = h1 - h0
                    nc.tensor.matmul(q2v[0:S, 0:WL], QT2[:hc, ci, :], vtall[:hc, ci, :],
                                     start=(ci == 0), stop=(ci == 2))
                invnqAll = sb.tile([S, WL], f32, tag="invnqAll")
                nqw = sb.tile([S, WL], f32, tag="nqw")
                nc.scalar.sqrt(nqw[:], q2v[0:S, 0:WL])
                scrW2 = sb.tile([S, WL], f32, tag="scrW2")
                nc.vector.reciprocal_approx_accurate(invnqAll[:], nqw[:], scrW2[:])

                # ============ FULL matching (w = d) ============
                if "fu" in en:
                    w = d
                    tidx = S - 1 if d == 0 else 0
                    for (side_out, xT, yT, yT2, invnAll) in (
                            (opt, PT, QT, QT2, invnpAll),
                            (oqt, QT, PT, PT2, invnqAll)):
                        g = sb.tile([128, 3, S], f32, tag="gf")
                        for ci in range(3):
                            nc.vector.tensor_scalar_mul(
                                g[:, ci, :], xT[:, ci, :], yT[:, ci, tidx:tidx + 1])
                        nums = ps.tile([128, 192], f32, tag="t")
                        for ci, (h0, h1) in enumerate(CH):
                            hc = h1 - h0
                            nc.tensor.matmul(nums[0:S, 0:L], g[:hc, ci, :], vts(ci, w),
                                             start=(ci == 0), stop=(ci == 2))
                        ql = ps.tile([128, 192], f32, tag="t")
                        for ci, (h0, h1) in enumerate(CH):
                            hc = h1 - h0
                            nc.tensor.matmul(ql[0:1, 0:L], yT2[:hc, ci, tidx:tidx + 1],
                                             vts(ci, w), start=(ci == 0), stop=(ci == 2))
                        qln = sb.tile([1, L], f32, tag="qln")
                        nc.scalar.sqrt(qln[:], ql[0:1, 0:L])
                        invql = sb.tile([1, L], f32, tag="invql")
                        scr1 = sb.tile([1, L], f32, tag="scr1")
                        nc.vector.reciprocal_approx_accurate(invql[:], qln[:], scr1[:])
                        qlr = ps.tile([128, 192], f32, tag="t")
                        nc.tensor.matmul(qlr[0:S, 0:L], ones[0:1, 0:S], invql[0:1, :],
                                         start=True, stop=True)
                        c1 = sb.tile([S, L], f32, tag="fc1")
                        nc.vector.tensor_mul(c1[:], invnAll[:, w * L:(w + 1) * L],
                                             qlr[0:S, 0:L])
                        c2 = sb.tile([S, L], f32, tag="fc2")
                        nc.vector.tensor_scalar_min(c2[:], c1[:], 1e8)
                        nc.vector.tensor_mul(side_out[:, w * L:(w + 1) * L],
                                             nums[0:S, 0:L], c2[:])

                # ============ MAXPOOL matching (w = 2 + d) ============
                if "mp" in en:
                    w = 2 + d
                    # transposed weighted norms for the "inner" side, then
                    # DRAM-roundtrip flatten + broadcast to 64 partitions.
                    reps = {}
                    for (nm, xT2) in (("q", QT2), ("p", PT2)):
                        nvt = ps.tile([128, 192], f32, tag="t")
                        for ci, (h0, h1) in enumerate(CH):
                            hc = h1 - h0
                            nc.tensor.matmul(nvt[0:L, 0:S], vts(ci, w), xT2[:hc, ci, :],
                                             start=(ci == 0), stop=(ci == 2))
                        nT = sb.tile([L, S], f32, tag="nT")
                        nc.scalar.sqrt(nT[:], nvt[0:L, 0:S])
                        invT = sb.tile([L, S], f32, tag="invT")
                        scrT = sb.tile([L, S], f32, tag="scrT")
                        nc.vector.reciprocal_approx_accurate(invT[:], nT[:], scrT[:])
                        scrd = dram.tile([L, S], f32, tag="nTd")
                        nc.sync.dma_start(scrd[:], invT[:])
                        repsb = sb.tile([S, L * S], f32, tag="invR" + nm)
                        src = bass.AP(tensor=scrd.tensor, offset=scrd.offset,
                                      ap=[[0, S], [1, L * S]])
                        nc.sync.dma_start(repsb[:], src)
                        reps[nm] = repsb

                    for (side_out, statT, rhs_srcT, invR, invnAll) in (
                            (opt, PT, QT, reps["q"], invnpAll),
                            (oqt, QT, PT, reps["p"], invnqAll)):
                        rhsall = sbx.tile([128, 3, L, S], f32, tag="rhsall")
                        for ci, (h0, h1) in enumerate(CH):
                            hc = h1 - h0
                            in0 = _bc_mid(bass, rhs_srcT[:, ci, :], L, 0)
                            in1 = _bc_mid(bass, vtall[:, ci, w * L:(w + 1) * L], S, 1)
                            eng = nc.vector if ci != 2 else nc.gpsimd
                            eng.tensor_mul(rhsall[:, ci], in0, in1)
                        dl = psdl.tile([S, L * S], f32, tag="dl")
                        for ci, (h0, h1) in enumerate(CH):
                            hc = h1 - h0
                            lt = statT[:hc, ci, :]
                            rh = flat3(rhsall, hc, ci)
                            for (n0, n1) in ((0, 512), (512, 1024), (1024, 1280)):
                                if F32R_MM:
                                    nc.tensor.matmul(dl[:, n0:n1], r32(lt),
                                                     r32(rh[:, n0:n1]),
                                                     start=(ci == 0), stop=(ci == 2))
                                else:
                                    nc.tensor.matmul(dl[:, n0:n1], lt, rh[:, n0:n1],
                                                     start=(ci == 0), stop=(ci == 2))
                        dsc = sb.tile([S, L * S], f32, tag="dsc")
                        nc.vector.tensor_mul(dsc[:], dl[:], invR[:])
                        mx = sb.tile([S, L], f32, tag="mx")
                        dsc3 = bass.AP(tensor=dsc.tensor, offset=dsc.offset,
                                       ap=[list(dsc.ap[0]), [S, L], [1, S]])
                        nc.vector.reduce_max(out=mx[:], in_=dsc3, axis=AX.X)
                        nc.vector.tensor_mul(side_out[:, w * L:(w + 1) * L], mx[:],
                                             invnAll[:, w * L:(w + 1) * L])

                # ============ ATTENTIVE-MEAN matching (w = 4 + d) ============
                if "am" in en:
                    w = 4 + d
                    rs = sb.tile([S, 1], f32, tag="rs")
                    nc.vector.reduce_sum(out=rs[:], in_=Cs[:], axis=AX.X)
                    sgr = sb.tile([S, 1], f32, tag="sgr")
                    nc.scalar.sign(sgr[:], rs[:])
                    cs_ = sb.tile([S, 1], f32, tag="cs_")
                    nc.vector.reduce_sum(out=cs_[:], in_=Ct[:], axis=AX.X)
                    sgc = sb.tile([S, 1], f32, tag="sgc")
                    nc.scalar.sign(sgc[:], cs_[:])

                    for (side_out, nat, cmat, statT, invnAll, sg) in (
                            (opt, Q, Ct, PT, invnpAll, sgr),
                            (oqt, P, Cs, QT, invnqAll, sgc)):
                        yvu = ps.tile([128, 192], f32, tag="t")
                        for ci, (h0, h1) in enumerate(CH):
                            hc = h1 - h0
                            nc.tensor.matmul(yvu[:hc, ci * S:(ci + 1) * S],
                                             nat[:, h0:h1], cmat[:],
                                             start=True, stop=True)
                        yvs = sb.tile([128, 3, S], f32, tag="yvs")
                        nc.scalar.copy(yvs[:], bass.AP(
                            tensor=yvu.tensor, offset=yvu.offset,
                            ap=[list(yvu.ap[0]), [S, 3], [1, S]]))
                        g = sb.tile([128, 3, S], f32, tag="gam")
                        nc.vector.tensor_mul(g[:], statT[:], yvs[:])
                        nums = ps.tile([128, 192], f32, tag="t")
                        for ci, (h0, h1) in enumerate(CH):
                            hc = h1 - h0
                            nc.tensor.matmul(nums[0:S, 0:L], g[:hc, ci, :], vts(ci, w),
                                             start=(ci == 0), stop=(ci == 2))
                        mp_tail(nums, yvs, w, invnAll, side_out, sgn=sg, clamp=True)

                # ============ ATTENTIVE-MAX matching (w = 6 + d) ============
                if "ax" in en:
                    w = 6 + d
                    csd = dram.tile([S, S], f32, tag="csd")
                    nc.sync.dma_start(csd[:], Cs[:])
                    csf = sb.tile([1, S * S], f32, tag="csf")
                    nc.sync.dma_start(csf[:], bass.AP(
                        tensor=csd.tensor, offset=csd.offset, ap=[[0, 1], [1, S * S]]))
                    ctd = dram.tile([S, S], f32, tag="ctd")
                    nc.sync.dma_start(ctd[:], Ct[:])
                    ctf = sb.tile([1, S * S], f32, tag="ctf")
                    nc.sync.dma_start(ctf[:], bass.AP(
                        tensor=ctd.tensor, offset=ctd.offset, ap=[[0, 1], [1, S * S]]))

                    nblk = (S * S) // REP_BLK
                    sblk = REP_BLK // S
                    res = {}
                    for (nm, flat, srcT) in (("q", csf, QT), ("p", ctf, PT)):
                        ymaxT = sb3.tile([128, 3, S], f32, tag="ymaxT" + nm)
                        for blk in range(nblk):
                            rep = psrep.tile([128, REP_BLK], f32, tag="rep")
                            for (n0, n1) in ((0, 512), (512, REP_BLK)):
                                fsl = flat[0:1, blk * REP_BLK + n0:blk * REP_BLK + n1]
                                if F32R_REP:
                                    nc.tensor.matmul(rep[:, n0:n1], r32(ones[0:1, :]),
                                                     r32(fsl), start=True, stop=True)
                                else:
                                    nc.tensor.matmul(rep[:, n0:n1], ones[0:1, :], fsl,
                                                     start=True, stop=True)
                            repsb = sbx.tile([128, REP_BLK], f32, tag="repsb")
                            nc.scalar.copy(repsb[:], rep[:])
                            rep3 = bass.AP(tensor=repsb.tensor, offset=repsb.offset,
                                           ap=[list(repsb.ap[0]), [S, sblk], [1, S]])
                            for ci in range(3):
                                X = sbX.tile([128, REP_BLK], f32, tag="X")
                                x3 = bass.AP(tensor=X.tensor, offset=X.offset,
                                             ap=[list(X.ap[0]), [S, sblk], [1, S]])
                                in0 = _bc_mid(bass, srcT[:, ci, :], sblk, 0)
                                bld_eng = nc.gpsimd if (blk * 3 + ci) % 5 < 3 else nc.vector
                                bld_eng.tensor_mul(x3, in0, rep3)
                                nc.vector.reduce_max(
                                    out=ymaxT[:, ci, blk * sblk:(blk + 1) * sblk],
                                    in_=x3, axis=AX.X)
                        res[nm] = ymaxT

                    for (side_out, statT, ymaxT, invnAll) in (
                            (opt, PT, res["q"], invnpAll),
                            (oqt, QT, res["p"], invnqAll)):
                        g = sb.tile([128, 3, S], f32, tag="gax")
                        nc.vector.tensor_mul(g[:], statT[:], ymaxT[:])
                        nums = ps.tile([128, 192], f32, tag="t")
                        for ci, (h0, h1) in enumerate(CH):
                            hc = h1 - h0
                            nc.tensor.matmul(nums[0:S, 0:L], g[:hc, ci, :], vts(ci, w),
                                             start=(ci == 0), stop=(ci == 2))
                        mp_tail(nums, ymaxT, w, invnAll, side_out, sgn=None, clamp=True)

            nc.sync.dma_start(op_d[b], opt[:])
            nc.sync.dma_start(oq_d[b], oqt[:])

    nc.compile()
    return nc


def _get_nc(nb=NB, en=("fu", "mp", "am", "ax")):
    key = (nb, tuple(en))
    if key not in _CACHE:
        _CACHE[key] = _build(nb, en)
    return _CACHE[key]


def _run(p, q, W, nb=NB, en=("fu", "mp", "am", "ax"), trace=False):
    from concourse.bass_utils import run_bass_kernel_spmd
    nc = _get_nc(nb, en)
    B = p.shape[0]
    ncores = B // nb
    assert ncores == NCORES and B == nb * NCORES
    in_maps = []
    for c in range(NCORES):
        in_maps.append({
            "p": np.ascontiguousarray(p[c * nb:(c + 1) * nb]),
            "q": np.ascontiguousarray(q[c * nb:(c + 1) * nb]),
            "W": np.ascontiguousarray(W),
        })
    r = run_bass_kernel_spmd(nc, in_maps, core_ids=list(range(NCORES)), trace=trace)
    if trace:
        print("HW exec time:", r.exec_time_ns, "ns")
        print("trace:", r.instructions_and_trace[1] if r.instructions_and_trace else None)
    mv_p = np.concatenate([r.results[c]["op"] for c in range(NCORES)], axis=0)
    mv_q = np.concatenate([r.results[c]["oq"] for c in range(NCORES)], axis=0)
    return mv_p, mv_q


def kernel(p, q, W):
    p = np.asarray(p, dtype=np.float32)
    q = np.asarray(q, dtype=np.float32)
    W = np.asarray(W, dtype=np.float32)
    return _run(p, q, W)

